# revision 1
# baseline (speedup 1.0000x reference)
"""Trainium2 Bass kernel for CommunityPassing (segment mean + gather).

Algorithm (8 NeuronCores, data-parallel over nodes):
  host: shard x/community over 8 cores along the node axis; within each
        shard, stably sort node indices by community id and pack them into
        128-row tiles grouped by community "chunk" (128 communities per
        chunk, 8 chunks for 1000 communities). Pad each (core, chunk)
        block to a shared tile count so all cores run one SPMD program.
  dev:  phase 1 - stream sorted x tiles; build a per-tile one-hot
        selection matrix B[node, local_comm] with a DVE is_equal against
        an iota row; matmul B^T @ x_tile accumulating into a PSUM tile
        per community chunk -> per-core partial community sums.
        AllReduce the [1024, 256] partial sums across the 8 cores,
        multiply by host-computed 1/count, write the [1024, 256]
        community-mean table to DRAM.
        phase 2 - dma_gather rows of the table with the original-order
        community ids (int16) and stream the result to the output.
  host: concatenate the 8 output shards.
"""

import os
import sys

import numpy as np

for _p in ("/opt/trn_rl_repo", "/opt/pypackages"):
    if _p not in sys.path and os.path.isdir(_p):
        sys.path.append(_p)

# Problem constants (hardcoded per the task contract).
N_FULL = 500000
F = 256
NUM_COMMS = 1000
EPS = 1e-12
M = 8               # cores
P = 128             # partitions
NC_CHUNKS = 8       # community chunks of 128 (8*128 = 1024 >= 1000)
GATHER_BATCH = 2048  # rows per dma_gather (multiple of 128)
XB = 8              # x tiles per streaming DMA (8 * 128KB = 1MB)
JB = GATHER_BATCH // P

# Stash of the most recent run's BassKernelResults (for test harnesses).
LAST_RESULTS = None


def _host_prep(x, community):
    """Build per-core device inputs. Returns (in_maps, plan)."""
    x = np.ascontiguousarray(np.asarray(x, dtype=np.float32))
    community = np.asarray(community).astype(np.int64)
    n = x.shape[0]
    assert n % M == 0
    nl = n // M

    comm_sh = community.reshape(M, nl)
    perms = np.argsort(comm_sh, axis=1, kind="stable")
    comm_sorted = np.take_along_axis(comm_sh, perms, axis=1)

    # per (core, chunk) node counts
    chunk_ids = comm_sorted >> 7  # // 128
    cnts = np.zeros((M, NC_CHUNKS), dtype=np.int64)
    for m in range(M):
        bc = np.bincount(chunk_ids[m], minlength=NC_CHUNKS)
        cnts[m] = bc[:NC_CHUNKS]
    t_k = np.maximum(1, -(-cnts.max(axis=0) // P))  # ceil, shared by all cores
    t_total = int(t_k.sum())
    chunk_of_tile = np.repeat(np.arange(NC_CHUNKS), t_k)
    tile_off = np.concatenate([[0], np.cumsum(t_k)])  # tile index base per chunk

    # counts -> 1/max(cnt, eps), [p, k] layout (community id = k*128 + p)
    cnt_full = np.bincount(community, minlength=NUM_COMMS).astype(np.float32)
    inv_pad = np.zeros((NC_CHUNKS * P,), np.float32)
    inv_pad[:NUM_COMMS] = 1.0 / np.maximum(cnt_full, np.float32(EPS))
    invc = np.ascontiguousarray(inv_pad.reshape(NC_CHUNKS, P).T)  # [128, 8]

    iota = np.ascontiguousarray(
        np.tile(np.arange(P, dtype=np.float32), (P, 1))
    )  # [128, 128], each row 0..127
    import ml_dtypes

    ident = np.eye(P).astype(ml_dtypes.bfloat16)

    in_maps = []
    origs = []
    for m in range(M):
        x_m = x[m * nl : (m + 1) * nl]
        xs = np.zeros((t_total * P, F), dtype=np.float32)
        locid = np.full((t_total * P,), -1.0, dtype=np.float32)
        orig = np.full((t_total * P,), -1, dtype=np.int64)
        start = 0
        for k in range(NC_CHUNKS):
            c = int(cnts[m, k])
            row = int(tile_off[k]) * P
            sel = perms[m, start : start + c]
            xs[row : row + c] = x_m[sel]
            orig[row : row + c] = sel
            locid[row : row + c] = comm_sorted[m, start : start + c] - k * P
            start += c
        locid_t = np.ascontiguousarray(locid.reshape(t_total, P).T)  # [128, T]
        origs.append(orig)

        xs_hi = xs.astype(ml_dtypes.bfloat16)
        xs_lo = (xs - xs_hi.astype(np.float32)).astype(ml_dtypes.bfloat16)
        in_maps.append(
            {
                "xs_hi": xs_hi,
                "xs_lo": xs_lo,
                "locid": locid_t,
                "iota": iota,
                "ident": ident,
                "invc": invc,
            }
        )

    plan = {
        "nl": nl,
        "t_k": [int(v) for v in t_k],
        "t_total": t_total,
        "chunk_of_tile": [int(v) for v in chunk_of_tile],
        "tile_off": [int(v) for v in tile_off],
        "origs": origs,
    }
    return in_maps, plan


def _build_program(plan, use_collective=True, use_gather=True):
    from concourse import bacc, mybir, tile

    t_total = plan["t_total"]
    chunk_of_tile = plan["chunk_of_tile"]
    tile_off = plan["tile_off"]

    dt = mybir.dt
    nc = bacc.Bacc("TRN2", target_bir_lowering=False, debug=False, num_devices=M)

    xs_hi = nc.dram_tensor("xs_hi", [t_total * P, F], dt.bfloat16, kind="ExternalInput")
    xs_lo = nc.dram_tensor("xs_lo", [t_total * P, F], dt.bfloat16, kind="ExternalInput")
    locid = nc.dram_tensor("locid", [P, t_total], dt.float32, kind="ExternalInput")
    iota = nc.dram_tensor("iota", [P, P], dt.float32, kind="ExternalInput")
    ident = nc.dram_tensor("ident", [P, P], dt.bfloat16, kind="ExternalInput")
    invc = nc.dram_tensor("invc", [P, NC_CHUNKS], dt.float32, kind="ExternalInput")
    out = nc.dram_tensor("out", [t_total * P, F], dt.float32, kind="ExternalOutput")

    xs_hi_view = xs_hi.ap().rearrange("(t p) f -> p t f", p=P)  # [128, T, 256]
    xs_lo_view = xs_lo.ap().rearrange("(t p) f -> p t f", p=P)

    with tile.TileContext(nc) as tc:
        with (
            tc.tile_pool(name="const", bufs=1) as constp,
            tc.tile_pool(name="xsp", bufs=3) as xsp,
            tc.tile_pool(name="bp", bufs=6) as bp,
            tc.tile_pool(name="acc", bufs=1) as accp,
            tc.tile_pool(name="psum", bufs=2, space="PSUM") as psp,
            tc.tile_pool(name="dram", bufs=1, space="DRAM") as dramp,
        ):
            iota_t = constp.tile([P, P], dt.float32)
            nc.sync.dma_start(out=iota_t[:], in_=iota.ap())
            ident_t = constp.tile([P, P], dt.bfloat16)
            nc.sync.dma_start(out=ident_t[:], in_=ident.ap())
            locid_t = constp.tile([P, t_total], dt.float32)
            nc.sync.dma_start(out=locid_t[:], in_=locid.ap())
            invc_t = constp.tile([P, NC_CHUNKS], dt.float32)
            nc.sync.dma_start(out=invc_t[:], in_=invc.ap())

            comm_sum = accp.tile([P, NC_CHUNKS * F], dt.float32)

            # ---- phase 1: streamed one-hot matmul segment sums ----
            xsb_hi = None
            xsb_lo = None
            bstart = 0
            psum_t = None
            for t in range(t_total):
                if t % XB == 0:
                    bstart = t
                    w = min(XB, t_total - t)
                    xsb_hi = xsp.tile([P, XB * F], dt.bfloat16, tag="xsbh")
                    nc.sync.dma_start(
                        out=xsb_hi[:, : w * F].rearrange("p (b f) -> p b f", b=w),
                        in_=xs_hi_view[:, t : t + w, :],
                    )
                    xsb_lo = xsp.tile([P, XB * F], dt.bfloat16, tag="xsbl")
                    nc.sync.dma_start(
                        out=xsb_lo[:, : w * F].rearrange("p (b f) -> p b f", b=w),
                        in_=xs_lo_view[:, t : t + w, :],
                    )
                k = chunk_of_tile[t]
                first = t == tile_off[k]
                last = t == tile_off[k + 1] - 1
                if first:
                    psum_t = psp.tile([P, F], dt.float32)
                b_t = bp.tile([P, P], dt.bfloat16, tag="b")
                nc.vector.tensor_scalar(
                    b_t[:],
                    iota_t[:],
                    locid_t[:, t : t + 1],
                    None,
                    mybir.AluOpType.is_equal,
                )
                j = t - bstart
                nc.tensor.matmul(
                    psum_t[:],
                    lhsT=b_t[:],
                    rhs=xsb_hi[:, j * F : (j + 1) * F],
                    start=first,
                    stop=False,
                )
                nc.tensor.matmul(
                    psum_t[:],
                    lhsT=b_t[:],
                    rhs=xsb_lo[:, j * F : (j + 1) * F],
                    start=False,
                    stop=last,
                )
                if last:
                    nc.vector.tensor_copy(
                        out=comm_sum[:, k * F : (k + 1) * F], in_=psum_t[:]
                    )

            # ---- all-reduce partial sums, scale by 1/count, write table ----
            ar_in = dramp.tile([P, NC_CHUNKS * F], dt.float32)
            ar_out = dramp.tile([P, NC_CHUNKS * F], dt.float32)
            nc.sync.dma_start(out=ar_in, in_=comm_sum[:])
            if use_collective:
                nc.gpsimd.collective_compute(
                    "AllReduce",
                    mybir.AluOpType.add,
                    replica_groups=[list(range(M))],
                    ins=[ar_in.opt()],
                    outs=[ar_out.opt()],
                )
            else:
                nc.sync.dma_start(out=ar_out, in_=ar_in)
            mean_sb = accp.tile([P, NC_CHUNKS * F], dt.float32)
            nc.sync.dma_start(out=mean_sb[:], in_=ar_out)
            for k in range(NC_CHUNKS):
                nc.vector.tensor_scalar(
                    mean_sb[:, k * F : (k + 1) * F],
                    mean_sb[:, k * F : (k + 1) * F],
                    invc_t[:, k : k + 1],
                    None,
                    mybir.AluOpType.mult,
                )

            # ---- phase 2: broadcast means back to (sorted) nodes ----
            # out_tile[node, f] = B[node, comm] @ mean_chunk[comm, f];
            # matmul wants lhsT = B^T, produced by a PE transpose.
            # fp32 matmul streams at 1/4 rate, so split the mean into two
            # bf16 limbs (hi + residual) and run two full-rate bf16 matmuls
            # accumulating in fp32 PSUM (~16-bit-exact result).
            mean_hi = accp.tile([P, NC_CHUNKS * F], dt.bfloat16)
            mean_lo = accp.tile([P, NC_CHUNKS * F], dt.bfloat16)
            mean_rest = accp.tile([P, NC_CHUNKS * F], dt.float32)
            nc.vector.tensor_copy(out=mean_hi[:], in_=mean_sb[:])
            nc.vector.tensor_copy(out=mean_rest[:], in_=mean_hi[:])
            nc.vector.tensor_tensor(
                out=mean_rest[:],
                in0=mean_sb[:],
                in1=mean_rest[:],
                op=mybir.AluOpType.subtract,
            )
            nc.vector.tensor_copy(out=mean_lo[:], in_=mean_rest[:])
            out_view = out.ap().rearrange("(t p) f -> p t f", p=P)
            with (
                tc.tile_pool(name="pst", bufs=2, space="PSUM") as pst,
                tc.tile_pool(name="pso", bufs=2, space="PSUM") as pso,
                tc.tile_pool(name="btp", bufs=4) as btp,
                tc.tile_pool(name="outp", bufs=3) as outp,
            ):
                if not use_gather:
                    t_total2 = 0
                else:
                    t_total2 = t_total
                outsb = None
                for t in range(t_total2):
                    if t % XB == 0:
                        outsb = outp.tile([P, XB * F], dt.float32, tag="outsb")
                    k = chunk_of_tile[t]
                    b2 = bp.tile([P, P], dt.bfloat16, tag="b2")
                    nc.vector.tensor_scalar(
                        b2[:],
                        iota_t[:],
                        locid_t[:, t : t + 1],
                        None,
                        mybir.AluOpType.is_equal,
                    )
                    bt_ps = pst.tile([P, P], dt.bfloat16)
                    nc.tensor.transpose(out=bt_ps[:], in_=b2[:], identity=ident_t[:])
                    bt_sb = btp.tile([P, P], dt.bfloat16, tag="bt")
                    nc.scalar.copy(out=bt_sb[:], in_=bt_ps[:])
                    op_ps = pso.tile([P, F], dt.float32)
                    nc.tensor.matmul(
                        op_ps[:],
                        lhsT=bt_sb[:],
                        rhs=mean_hi[:, k * F : (k + 1) * F],
                        start=True,
                        stop=False,
                    )
                    nc.tensor.matmul(
                        op_ps[:],
                        lhsT=bt_sb[:],
                        rhs=mean_lo[:, k * F : (k + 1) * F],
                        start=False,
                        stop=True,
                    )
                    j = t % XB
                    nc.vector.tensor_copy(
                        out=outsb[:, j * F : (j + 1) * F], in_=op_ps[:]
                    )
                    if t % XB == XB - 1 or t == t_total2 - 1:
                        t0 = t - j
                        w = j + 1
                        nc.sync.dma_start(
                            out=out_view[:, t0 : t0 + w, :],
                            in_=outsb[:, : w * F].rearrange(
                                "p (b f) -> p b f", b=w
                            ),
                        )

    nc.compile()
    return nc


def kernel(x, community):
    global LAST_RESULTS
    from concourse.bass_utils import run_bass_kernel_spmd

    in_maps, plan = _host_prep(x, community)
    nc = _build_program(plan)
    res = run_bass_kernel_spmd(nc, in_maps, core_ids=list(range(M)))
    LAST_RESULTS = res
    nl = plan["nl"]
    outs = []
    for m in range(M):
        out_sorted = res.results[m]["out"]
        orig = plan["origs"][m]
        valid = orig >= 0
        out_m = np.empty((nl, F), dtype=np.float32)
        out_m[orig[valid]] = out_sorted[valid]
        outs.append(out_m)
    return np.concatenate(outs, axis=0)



# revision 4
# speedup vs baseline: 1.9192x; 1.9192x over previous
"""Trainium2 Bass kernel for CommunityPassing (segment mean + gather).

Algorithm (8 NeuronCores, data-parallel over nodes):
  host: shard x/community over 8 cores along the node axis; within each
        shard, stably sort node indices by community id and pack them into
        128-row tiles grouped by community "chunk" (128 communities per
        chunk, 8 chunks for 1000 communities). Pad each (core, chunk)
        block to a shared tile count so all cores run one SPMD program.
        Ship x pre-gathered in bf16, partition-major [128, T*F] layout so
        device DMAs are large and contiguous per partition.
  dev:  phase 1 - stream sorted x tiles; build a per-tile one-hot
        selection matrix B[node, local_comm] with a DVE is_equal against
        an iota row; matmul B^T @ x_tile accumulating into a PSUM tile
        per community chunk -> per-core partial community sums.
        AllReduce the [128, 8*256] partial sums across the 8 cores,
        multiply by host-computed 1/count -> bf16 community-mean table
        held in SBUF.
        phase 2 - per tile: rebuild B, transpose it on the PE, then two
        transpose-mode matmuls with lhsT = mean half (dense) and
        rhs = B^T (one-hot) produce out^T[f_half, node] in bf16 PSUM
        (transpose-mode matmuls require a one-hot rhs). Batched PSUM
        evacuation, one large DMA write per chunk in [f, node] layout.
  host: transpose out^T back, upcast bf16->fp32, inverse-permute.
"""

import os
import sys

import numpy as np

for _p in ("/opt/trn_rl_repo", "/opt/pypackages"):
    if _p not in sys.path and os.path.isdir(_p):
        sys.path.append(_p)

# Problem constants (hardcoded per the task contract).
N_FULL = 500000
F = 256
NUM_COMMS = 1000
EPS = 1e-12
M = 8               # cores
P = 128             # partitions
NC_CHUNKS = 8       # community chunks of 128 (8*128 = 1024 >= 1000)
XB = 16             # x tiles per streaming read DMA (16 * 64KB = 1MB)
W = 8               # tiles per phase-2 PSUM group

# Stash of the most recent run's BassKernelResults (for test harnesses).
LAST_RESULTS = None


def _host_prep(x, community):
    """Build per-core device inputs. Returns (in_maps, plan)."""
    import ml_dtypes

    x = np.ascontiguousarray(np.asarray(x, dtype=np.float32))
    community = np.asarray(community).astype(np.int64)
    n = x.shape[0]
    assert n % M == 0
    nl = n // M

    comm_sh = community.reshape(M, nl)
    perms = np.argsort(comm_sh, axis=1, kind="stable")
    comm_sorted = np.take_along_axis(comm_sh, perms, axis=1)

    # per (core, chunk) node counts
    chunk_ids = comm_sorted >> 7  # // 128
    cnts = np.zeros((M, NC_CHUNKS), dtype=np.int64)
    for m in range(M):
        bc = np.bincount(chunk_ids[m], minlength=NC_CHUNKS)
        cnts[m] = bc[:NC_CHUNKS]
    t_k = np.maximum(1, -(-cnts.max(axis=0) // P))  # ceil, shared by all cores
    t_total = int(t_k.sum())
    chunk_of_tile = np.repeat(np.arange(NC_CHUNKS), t_k)
    tile_off = np.concatenate([[0], np.cumsum(t_k)])  # tile index base per chunk

    # counts -> 1/max(cnt, eps), [p, k] layout (community id = k*128 + p)
    cnt_full = np.bincount(community, minlength=NUM_COMMS).astype(np.float32)
    inv_pad = np.zeros((NC_CHUNKS * P,), np.float32)
    inv_pad[:NUM_COMMS] = 1.0 / np.maximum(cnt_full, np.float32(EPS))
    invc = np.ascontiguousarray(inv_pad.reshape(NC_CHUNKS, P).T)  # [128, 8]

    iota = np.ascontiguousarray(
        np.tile(np.arange(P, dtype=ml_dtypes.bfloat16), (P, 1))
    )  # [128, 128], each row 0..127
    ident = np.eye(P).astype(ml_dtypes.bfloat16)

    in_maps = []
    origs = []
    for m in range(M):
        x_m = x[m * nl : (m + 1) * nl]
        locid = np.full((t_total * P,), -1.0, dtype=np.float32)
        orig = np.full((t_total * P,), -1, dtype=np.int64)
        start = 0
        for k in range(NC_CHUNKS):
            c = int(cnts[m, k])
            row = int(tile_off[k]) * P
            sel = perms[m, start : start + c]
            orig[row : row + c] = sel
            locid[row : row + c] = comm_sorted[m, start : start + c] - k * P
            start += c
        # partition-major [128, T] index grid; gather x directly into
        # [128, T, F] so the device reads contiguous per-partition lines
        orig_t = np.ascontiguousarray(orig.reshape(t_total, P).T)  # [128, T]
        xs = x_m[np.maximum(orig_t, 0)]  # [128, T, F] fp32
        xs[orig_t < 0] = 0.0
        xs_t = np.ascontiguousarray(
            xs.astype(ml_dtypes.bfloat16).reshape(P, t_total * F)
        )
        locid_t = np.ascontiguousarray(locid.reshape(t_total, P).T)  # [128, T] fp32
        origs.append(orig)

        in_maps.append(
            {
                "xs": xs_t,
                "locid": locid_t,
                "iota": iota,
                "ident": ident,
                "invc": invc,
            }
        )

    plan = {
        "nl": nl,
        "t_k": [int(v) for v in t_k],
        "t_total": t_total,
        "chunk_of_tile": [int(v) for v in chunk_of_tile],
        "tile_off": [int(v) for v in tile_off],
        "origs": origs,
    }
    return in_maps, plan


def _build_program(plan, use_collective=True):
    from concourse import bacc, mybir, tile

    t_total = plan["t_total"]
    t_k = plan["t_k"]
    chunk_of_tile = plan["chunk_of_tile"]
    tile_off = plan["tile_off"]
    max_tk = max(t_k)

    dt = mybir.dt
    nc = bacc.Bacc("TRN2", target_bir_lowering=False, debug=False, num_devices=M)

    xs = nc.dram_tensor("xs", [P, t_total * F], dt.bfloat16, kind="ExternalInput")
    locid = nc.dram_tensor("locid", [P, t_total], dt.float32, kind="ExternalInput")
    iota = nc.dram_tensor("iota", [P, P], dt.bfloat16, kind="ExternalInput")
    ident = nc.dram_tensor("ident", [P, P], dt.bfloat16, kind="ExternalInput")
    invc = nc.dram_tensor("invc", [P, NC_CHUNKS], dt.float32, kind="ExternalInput")
    out = nc.dram_tensor("out", [2 * P, t_total * P], dt.bfloat16, kind="ExternalOutput")

    with tile.TileContext(nc) as tc:
        with (
            tc.tile_pool(name="const", bufs=1) as constp,
            tc.tile_pool(name="xsp", bufs=3) as xsp,
            tc.tile_pool(name="bp", bufs=4) as bp,
            tc.tile_pool(name="acc", bufs=1) as accp,
            tc.tile_pool(name="dram", bufs=1, space="DRAM") as dramp,
        ):
            iota_t = constp.tile([P, P], dt.bfloat16)
            nc.sync.dma_start(out=iota_t[:], in_=iota.ap())
            ident_t = constp.tile([P, P], dt.bfloat16)
            nc.sync.dma_start(out=ident_t[:], in_=ident.ap())
            locid_t = constp.tile([P, t_total], dt.float32)
            nc.sync.dma_start(out=locid_t[:], in_=locid.ap())
            invc_t = constp.tile([P, NC_CHUNKS], dt.float32)
            nc.sync.dma_start(out=invc_t[:], in_=invc.ap())

            comm_sum = accp.tile([P, NC_CHUNKS * F], dt.float32)

            # ---- phase 1: streamed one-hot matmul segment sums ----
            with tc.tile_pool(name="ps1", bufs=2, space="PSUM") as ps1p:
                xsb = None
                psum_t = None
                for t in range(t_total):
                    if t % XB == 0:
                        w = min(XB, t_total - t)
                        xsb = xsp.tile([P, XB * F], dt.bfloat16, tag="xsb")
                        nc.sync.dma_start(
                            out=xsb[:, : w * F],
                            in_=xs.ap()[:, t * F : (t + w) * F],
                        )
                    k = chunk_of_tile[t]
                    first = t == tile_off[k]
                    last = t == tile_off[k + 1] - 1
                    if first:
                        psum_t = ps1p.tile([P, F], dt.float32)
                    b_t = bp.tile([P, P], dt.bfloat16, tag="b")
                    nc.vector.tensor_scalar(
                        b_t[:],
                        iota_t[:],
                        locid_t[:, t : t + 1],
                        None,
                        mybir.AluOpType.is_equal,
                    )
                    j = t % XB
                    nc.tensor.matmul(
                        psum_t[:],
                        lhsT=b_t[:],
                        rhs=xsb[:, j * F : (j + 1) * F],
                        start=first,
                        stop=last,
                    )
                    if last:
                        nc.vector.tensor_copy(
                            out=comm_sum[:, k * F : (k + 1) * F], in_=psum_t[:]
                        )

            # ---- all-reduce partial sums, scale by 1/count -> bf16 table ----
            ar_in = dramp.tile([P, NC_CHUNKS * F], dt.float32)
            ar_out = dramp.tile([P, NC_CHUNKS * F], dt.float32)
            nc.sync.dma_start(out=ar_in, in_=comm_sum[:])
            if use_collective:
                nc.gpsimd.collective_compute(
                    "AllReduce",
                    mybir.AluOpType.add,
                    replica_groups=[list(range(M))],
                    ins=[ar_in.opt()],
                    outs=[ar_out.opt()],
                )
            else:
                nc.sync.dma_start(out=ar_out, in_=ar_in)
            mean_f32 = accp.tile([P, NC_CHUNKS * F], dt.float32)
            nc.sync.dma_start(out=mean_f32[:], in_=ar_out)
            mean_bf = accp.tile([P, NC_CHUNKS * F], dt.bfloat16)
            for k in range(NC_CHUNKS):
                nc.vector.tensor_scalar(
                    mean_bf[:, k * F : (k + 1) * F],
                    mean_f32[:, k * F : (k + 1) * F],
                    invc_t[:, k : k + 1],
                    None,
                    mybir.AluOpType.mult,
                )

            # ---- phase 2: broadcast means back to (sorted) nodes ----
            # per tile: bT = PE-transpose(B); two transpose-mode matmuls
            # (lhsT = dense mean half, rhs = one-hot bT) produce
            # out^T[f_half, node] in bf16 PSUM. Batched evacuation split
            # across DVE/Scalar, one big DMA per chunk.
            out_v = out.ap().rearrange("(h p) n -> p h n", p=P)
            with (
                tc.tile_pool(name="pst", bufs=2, space="PSUM") as pstp,
                tc.tile_pool(name="pso", bufs=2, space="PSUM") as psop,
                tc.tile_pool(name="btp", bufs=3) as btp,
                tc.tile_pool(name="outp", bufs=2) as outp,
            ):
                gidx = 0
                for k in range(NC_CHUNKS):
                    k0, k1 = tile_off[k], tile_off[k + 1]
                    outsb = outp.tile(
                        [P, 2 * max_tk * P], dt.bfloat16, tag="outsb"
                    )
                    for g0 in range(k0, k1, W):
                        g1 = min(g0 + W, k1)
                        w = g1 - g0
                        pst_t = pstp.tile([P, W * P], dt.bfloat16, tag="pst")
                        for j in range(w):
                            t = g0 + j
                            b2 = bp.tile([P, P], dt.bfloat16, tag="b2")
                            nc.vector.tensor_scalar(
                                b2[:],
                                iota_t[:],
                                locid_t[:, t : t + 1],
                                None,
                                mybir.AluOpType.is_equal,
                            )
                            nc.tensor.transpose(
                                out=pst_t[:, j * P : (j + 1) * P],
                                in_=b2[:],
                                identity=ident_t[:],
                            )
                        bt_sb = btp.tile([P, W * P], dt.bfloat16, tag="bt")
                        nc.scalar.copy(
                            out=bt_sb[:, : w * P], in_=pst_t[:, : w * P]
                        )
                        pso_lo = psop.tile([P, W * P], dt.bfloat16, tag="psolo")
                        pso_hi = psop.tile([P, W * P], dt.bfloat16, tag="psohi")
                        for j in range(w):
                            nc.tensor.matmul(
                                pso_lo[:, j * P : (j + 1) * P],
                                lhsT=mean_bf[:, k * F : k * F + P],
                                rhs=bt_sb[:, j * P : (j + 1) * P],
                                start=True,
                                stop=True,
                                is_transpose=True,
                            )
                        for j in range(w):
                            nc.tensor.matmul(
                                pso_hi[:, j * P : (j + 1) * P],
                                lhsT=mean_bf[:, k * F + P : (k + 1) * F],
                                rhs=bt_sb[:, j * P : (j + 1) * P],
                                start=True,
                                stop=True,
                                is_transpose=True,
                            )
                        c0 = (g0 - k0) * P
                        hb = max_tk * P
                        if gidx % 2 == 0:
                            nc.vector.tensor_copy(
                                out=outsb[:, c0 : c0 + w * P],
                                in_=pso_lo[:, : w * P],
                            )
                            nc.scalar.copy(
                                out=outsb[:, hb + c0 : hb + c0 + w * P],
                                in_=pso_hi[:, : w * P],
                            )
                        else:
                            nc.scalar.copy(
                                out=outsb[:, c0 : c0 + w * P],
                                in_=pso_lo[:, : w * P],
                            )
                            nc.vector.tensor_copy(
                                out=outsb[:, hb + c0 : hb + c0 + w * P],
                                in_=pso_hi[:, : w * P],
                            )
                        gidx += 1
                    nc.sync.dma_start(
                        out=out_v[:, :, k0 * P : k1 * P],
                        in_=outsb.rearrange("p (h n) -> p h n", h=2)[
                            :, :, : (k1 - k0) * P
                        ],
                    )

    nc.compile()
    return nc


def kernel(x, community):
    global LAST_RESULTS
    from concourse.bass_utils import run_bass_kernel_spmd

    in_maps, plan = _host_prep(x, community)
    nc = _build_program(plan)
    res = run_bass_kernel_spmd(nc, in_maps, core_ids=list(range(M)))
    LAST_RESULTS = res
    nl = plan["nl"]
    outs = []
    for m in range(M):
        out_T = np.asarray(res.results[m]["out"])  # [256, T*128] bf16
        orig = plan["origs"][m]  # flat [T*128], node n = t*128+p
        valid = orig >= 0
        out_m = np.empty((nl, F), dtype=np.float32)
        out_m[orig[valid]] = out_T.T[valid].astype(np.float32)
        outs.append(out_m)
    return np.concatenate(outs, axis=0)


# revision 7
# speedup vs baseline: 2.2541x; 1.1745x over previous
"""Trainium2 Bass kernel for CommunityPassing (segment mean + gather).

Algorithm (8 NeuronCores, data-parallel over nodes):
  host: shard x/community over 8 cores along the node axis; within each
        shard, stably sort node indices by community id and pack them into
        128-row tiles grouped by community "chunk" (128 communities per
        chunk, 8 chunks for 1000 communities). Pad each (core, chunk)
        block to a shared tile count so all cores run one SPMD program.
        Ship x pre-gathered in bf16, partition-major [128, T*F] layout so
        device DMAs are large and contiguous per partition. Also ship the
        per-tile one-hot selector transpose bt[c, node] as fp8 (exact for
        0/1) so phase 2 needs no on-device one-hot build or transpose.
  dev:  phase 1 - stream sorted x tiles; build a per-tile one-hot
        B[node, local_comm] with a DVE is_equal against an iota row;
        matmul B^T @ x_tile accumulating into a PSUM tile per community
        chunk -> per-core partial community sums.
        AllReduce the partial sums in two halves (each overlapped with
        remaining compute), scale by host-computed 1/count on GpSimd ->
        bf16 community-mean table in SBUF.
        phase 2 - per tile: one matmul with lhsT = fp8 one-hot bt and
        rhs = bf16 mean chunk selects each node's community-mean row
        (fp32 PSUM). Batched PSUM evacuation alternating DVE/Scalar,
        large DMA writes in [128, T*F] layout.
  host: un-transpose, upcast bf16->fp32, inverse-permute the 8 shards.
"""

import os
import sys

import numpy as np

for _p in ("/opt/trn_rl_repo", "/opt/pypackages"):
    if _p not in sys.path and os.path.isdir(_p):
        sys.path.append(_p)

# Problem constants (hardcoded per the task contract).
N_FULL = 500000
F = 256
NUM_COMMS = 1000
EPS = 1e-12
M = 8               # cores
P = 128             # partitions
NC_CHUNKS = 8       # community chunks of 128 (8*128 = 1024 >= 1000)
XB = 32             # x tiles per streaming read DMA (32 * 64KB = 2MB)
W = 8               # tiles per phase-2 PSUM group
WG = 16             # tiles per phase-2 write DMA

# Stash of the most recent run's BassKernelResults (for test harnesses).
LAST_RESULTS = None


def _host_prep(x, community):
    """Build per-core device inputs. Returns (in_maps, plan)."""
    import ml_dtypes

    x = np.ascontiguousarray(np.asarray(x, dtype=np.float32))
    community = np.asarray(community).astype(np.int64)
    n = x.shape[0]
    assert n % M == 0
    nl = n // M

    comm_sh = community.reshape(M, nl)
    perms = np.argsort(comm_sh, axis=1, kind="stable")
    comm_sorted = np.take_along_axis(comm_sh, perms, axis=1)

    # per (core, chunk) node counts
    chunk_ids = comm_sorted >> 7  # // 128
    cnts = np.zeros((M, NC_CHUNKS), dtype=np.int64)
    for m in range(M):
        bc = np.bincount(chunk_ids[m], minlength=NC_CHUNKS)
        cnts[m] = bc[:NC_CHUNKS]
    t_k = np.maximum(1, -(-cnts.max(axis=0) // P))  # ceil, shared by all cores
    t_total = int(t_k.sum())
    chunk_of_tile = np.repeat(np.arange(NC_CHUNKS), t_k)
    tile_off = np.concatenate([[0], np.cumsum(t_k)])  # tile index base per chunk

    # counts -> 1/max(cnt, eps), [p, k] layout (community id = k*128 + p)
    cnt_full = np.bincount(community, minlength=NUM_COMMS).astype(np.float32)
    inv_pad = np.zeros((NC_CHUNKS * P,), np.float32)
    inv_pad[:NUM_COMMS] = 1.0 / np.maximum(cnt_full, np.float32(EPS))
    invc = np.ascontiguousarray(inv_pad.reshape(NC_CHUNKS, P).T)  # [128, 8]

    iota = np.ascontiguousarray(
        np.tile(np.arange(P, dtype=ml_dtypes.bfloat16), (P, 1))
    )  # [128, 128], each row 0..127

    in_maps = []
    origs = []
    for m in range(M):
        x_m = x[m * nl : (m + 1) * nl]
        locid = np.full((t_total * P,), -1.0, dtype=np.float32)
        orig = np.full((t_total * P,), -1, dtype=np.int64)
        start = 0
        for k in range(NC_CHUNKS):
            c = int(cnts[m, k])
            row = int(tile_off[k]) * P
            sel = perms[m, start : start + c]
            orig[row : row + c] = sel
            locid[row : row + c] = comm_sorted[m, start : start + c] - k * P
            start += c
        # partition-major [128, T] index grid; gather x directly into
        # [128, T, F] so the device reads contiguous per-partition lines
        orig_t = np.ascontiguousarray(orig.reshape(t_total, P).T)  # [128, T]
        xs = x_m[np.maximum(orig_t, 0)]  # [128, T, F] fp32
        xs[orig_t < 0] = 0.0
        xs_t = np.ascontiguousarray(
            xs.astype(ml_dtypes.bfloat16).reshape(P, t_total * F)
        )
        locid_t = np.ascontiguousarray(locid.reshape(t_total, P).T)  # [128, T] fp32

        # fp8 one-hot transpose: bt8[c, t*128+p] = 1 iff locid[t*128+p] == c
        bt8 = np.zeros((P, t_total * P), dtype=ml_dtypes.float8_e4m3fn)
        nidx = np.arange(t_total * P)
        valid = orig >= 0
        bt8[locid[valid].astype(np.int64), nidx[valid]] = 1.0

        origs.append(orig_t)
        in_maps.append(
            {
                "xs": xs_t,
                "locid": locid_t,
                "bt8": bt8,
                "iota": iota,
                "invc": invc,
            }
        )

    plan = {
        "nl": nl,
        "t_k": [int(v) for v in t_k],
        "t_total": t_total,
        "chunk_of_tile": [int(v) for v in chunk_of_tile],
        "tile_off": [int(v) for v in tile_off],
        "origs": origs,
    }
    return in_maps, plan


def _build_program(plan, use_collective=True):
    from concourse import bacc, mybir, tile

    t_total = plan["t_total"]
    t_k = plan["t_k"]
    chunk_of_tile = plan["chunk_of_tile"]
    tile_off = plan["tile_off"]
    max_tk = max(t_k)
    HALF = NC_CHUNKS // 2
    h0_end = tile_off[HALF]  # first tile of chunk 4

    dt = mybir.dt
    nc = bacc.Bacc("TRN2", target_bir_lowering=False, debug=False, num_devices=M)

    xs = nc.dram_tensor("xs", [P, t_total * F], dt.bfloat16, kind="ExternalInput")
    locid = nc.dram_tensor("locid", [P, t_total], dt.float32, kind="ExternalInput")
    bt8 = nc.dram_tensor("bt8", [P, t_total * P], dt.float8e4, kind="ExternalInput")
    iota = nc.dram_tensor("iota", [P, P], dt.bfloat16, kind="ExternalInput")
    invc = nc.dram_tensor("invc", [P, NC_CHUNKS], dt.float32, kind="ExternalInput")
    out = nc.dram_tensor("out", [P, t_total * F], dt.bfloat16, kind="ExternalOutput")

    with tile.TileContext(nc) as tc:
        with (
            tc.tile_pool(name="const", bufs=1) as constp,
            tc.tile_pool(name="xsp", bufs=3) as xsp,
            tc.tile_pool(name="bp", bufs=4) as bp,
            tc.tile_pool(name="acc", bufs=1) as accp,
            tc.tile_pool(name="btp", bufs=2) as btp,
            tc.tile_pool(name="dram", bufs=1, space="DRAM") as dramp,
        ):
            iota_t = constp.tile([P, P], dt.bfloat16)
            nc.sync.dma_start(out=iota_t[:], in_=iota.ap())
            locid_t = constp.tile([P, t_total], dt.float32)
            nc.sync.dma_start(out=locid_t[:], in_=locid.ap())
            invc_t = constp.tile([P, NC_CHUNKS], dt.float32)
            nc.sync.dma_start(out=invc_t[:], in_=invc.ap())

            comm_sum = accp.tile([P, NC_CHUNKS * F], dt.float32)
            mean_f32 = accp.tile([P, NC_CHUNKS * F], dt.float32)
            mean_bf = accp.tile([P, NC_CHUNKS * F], dt.bfloat16)
            HF = HALF * F  # columns per AR half
            ar_in0 = dramp.tile([P, HF], dt.float32)
            ar_in1 = dramp.tile([P, HF], dt.float32)
            ar_out0 = dramp.tile([P, HF], dt.float32)
            ar_out1 = dramp.tile([P, HF], dt.float32)
            ar_in = [ar_in0, ar_in1]
            ar_out = [ar_out0, ar_out1]

            def emit_ar_half(h):
                """All-reduce one half of comm_sum, scale on GpSimd."""
                lo, hi = h * HF, (h + 1) * HF
                # dmas on the Scalar HWDGE queue to keep the Sync read
                # stream unblocked; scaling on GpSimd to keep DVE rolling.
                nc.scalar.dma_start(out=ar_in[h], in_=comm_sum[:, lo:hi])
                if use_collective:
                    nc.gpsimd.collective_compute(
                        "AllReduce",
                        mybir.AluOpType.add,
                        replica_groups=[list(range(M))],
                        ins=[ar_in[h].opt()],
                        outs=[ar_out[h].opt()],
                    )
                else:
                    nc.scalar.dma_start(out=ar_out[h], in_=ar_in[h])
                nc.scalar.dma_start(out=mean_f32[:, lo:hi], in_=ar_out[h])
                for k in range(h * HALF, (h + 1) * HALF):
                    nc.gpsimd.tensor_scalar(
                        mean_bf[:, k * F : (k + 1) * F],
                        mean_f32[:, k * F : (k + 1) * F],
                        invc_t[:, k : k + 1],
                        None,
                        mybir.AluOpType.mult,
                    )

            # ---- phase 1: streamed one-hot matmul segment sums ----
            with tc.tile_pool(name="ps1", bufs=2, space="PSUM") as ps1p:
                xsb = None
                psum_t = None
                for t in range(t_total):
                    if t % XB == 0:
                        w = min(XB, t_total - t)
                        xsb = xsp.tile([P, XB * F], dt.bfloat16, tag="xsb")
                        nc.sync.dma_start(
                            out=xsb[:, : w * F],
                            in_=xs.ap()[:, t * F : (t + w) * F],
                        )
                    k = chunk_of_tile[t]
                    first = t == tile_off[k]
                    last = t == tile_off[k + 1] - 1
                    if first:
                        psum_t = ps1p.tile([P, F], dt.float32)
                    b_t = bp.tile([P, P], dt.bfloat16, tag="b")
                    nc.vector.tensor_scalar(
                        b_t[:],
                        iota_t[:],
                        locid_t[:, t : t + 1],
                        None,
                        mybir.AluOpType.is_equal,
                    )
                    j = t % XB
                    nc.tensor.matmul(
                        psum_t[:],
                        lhsT=b_t[:],
                        rhs=xsb[:, j * F : (j + 1) * F],
                        start=first,
                        stop=last,
                    )
                    if last:
                        nc.vector.tensor_copy(
                            out=comm_sum[:, k * F : (k + 1) * F], in_=psum_t[:]
                        )
                    if t == h0_end - 1:
                        emit_ar_half(0)
                emit_ar_half(1)

            # ---- phase 2: broadcast means back to (sorted) nodes ----
            # per tile: matmul(lhsT = fp8 one-hot bt, rhs = bf16 mean
            # chunk) -> fp32 PSUM; evacuate in W-tile groups alternating
            # DVE/Scalar; write WG-tile DMAs.
            with (
                tc.tile_pool(name="pso", bufs=2, space="PSUM") as psop,
                tc.tile_pool(name="outp", bufs=3) as outp,
            ):
                gidx = 0
                for k in range(NC_CHUNKS):
                    k0, k1 = tile_off[k], tile_off[k + 1]
                    bt_sb = btp.tile([P, max_tk * P], dt.float8e4, tag="btc")
                    nc.sync.dma_start(
                        out=bt_sb[:, : (k1 - k0) * P],
                        in_=bt8.ap()[:, k0 * P : k1 * P],
                    )
                    outsb = None
                    wg0 = k0
                    for g0 in range(k0, k1, W):
                        g1 = min(g0 + W, k1)
                        w = g1 - g0
                        if g0 == wg0:
                            outsb = outp.tile([P, WG * F], dt.bfloat16,
                                              tag="outsb")
                        pso_t = psop.tile([P, W * F], dt.float32, tag="pso")
                        for j in range(w):
                            t = g0 + j
                            nc.tensor.matmul(
                                pso_t[:, j * F : (j + 1) * F],
                                lhsT=bt_sb[:, (t - k0) * P : (t - k0 + 1) * P],
                                rhs=mean_bf[:, k * F : (k + 1) * F],
                                start=True,
                                stop=True,
                            )
                        c0 = (g0 - wg0) * F
                        if gidx % 2 == 0:
                            nc.vector.tensor_copy(
                                out=outsb[:, c0 : c0 + w * F],
                                in_=pso_t[:, : w * F],
                            )
                        else:
                            nc.scalar.copy(
                                out=outsb[:, c0 : c0 + w * F],
                                in_=pso_t[:, : w * F],
                            )
                        gidx += 1
                        if g1 - wg0 >= WG or g1 == k1:
                            nc.sync.dma_start(
                                out=out.ap()[:, wg0 * F : g1 * F],
                                in_=outsb[:, : (g1 - wg0) * F],
                            )
                            wg0 = g1

    nc.compile()
    return nc


def kernel(x, community):
    global LAST_RESULTS
    from concourse.bass_utils import run_bass_kernel_spmd

    in_maps, plan = _host_prep(x, community)
    nc = _build_program(plan)
    res = run_bass_kernel_spmd(nc, in_maps, core_ids=list(range(M)))
    LAST_RESULTS = res
    nl = plan["nl"]
    t_total = plan["t_total"]
    outs = []
    for m in range(M):
        out_t = np.asarray(res.results[m]["out"]).reshape(P, t_total, F)
        orig_t = plan["origs"][m]  # [128, T]
        valid = orig_t >= 0
        out_m = np.empty((nl, F), dtype=np.float32)
        out_m[orig_t[valid]] = out_t[valid].astype(np.float32)
        outs.append(out_m)
    return np.concatenate(outs, axis=0)


# revision 8
# speedup vs baseline: 2.4191x; 1.0732x over previous
"""Trainium2 Bass kernel for CommunityPassing (segment mean + gather).

Algorithm (8 NeuronCores, data-parallel over nodes):
  host: shard x/community over 8 cores along the node axis; within each
        shard, stably sort node indices by community id and pack them into
        128-row tiles grouped by community "chunk" (128 communities per
        chunk, 8 chunks for 1000 communities). Pad each (core, chunk)
        block to a shared tile count so all cores run one SPMD program.
        Ship x pre-gathered in bf16, partition-major [128, T*F] layout so
        device DMAs are large and contiguous per partition. Also ship the
        per-tile one-hot selector transpose bt[c, node] as fp8 (exact for
        0/1) so phase 2 needs no on-device one-hot build or transpose.
  dev:  phase 1 - stream sorted x tiles; build a per-tile one-hot
        B[node, local_comm] with a DVE is_equal against an iota row;
        matmul B^T @ x_tile accumulating into a PSUM tile per community
        chunk -> per-core partial community sums.
        AllReduce the partial sums in two halves (each overlapped with
        remaining compute), scale by host-computed 1/count on GpSimd ->
        bf16 community-mean table in SBUF.
        phase 2 - per tile: one matmul with lhsT = fp8 one-hot bt and
        rhs = bf16 mean chunk selects each node's community-mean row
        (fp32 PSUM). Batched PSUM evacuation alternating DVE/Scalar,
        large DMA writes in [128, T*F] layout.
  host: un-transpose, upcast bf16->fp32, inverse-permute the 8 shards.
"""

import os
import sys

import numpy as np

for _p in ("/opt/trn_rl_repo", "/opt/pypackages"):
    if _p not in sys.path and os.path.isdir(_p):
        sys.path.append(_p)

# Problem constants (hardcoded per the task contract).
N_FULL = 500000
F = 256
NUM_COMMS = 1000
EPS = 1e-12
M = 8               # cores
P = 128             # partitions
NC_CHUNKS = 8       # community chunks of 128 (8*128 = 1024 >= 1000)
XB = 32             # x tiles per streaming read DMA (32 * 64KB = 2MB)
W = 4               # tiles per phase-2 PSUM group
WG = 16             # tiles per phase-2 write DMA

# Stash of the most recent run's BassKernelResults (for test harnesses).
LAST_RESULTS = None


def _host_prep(x, community):
    """Build per-core device inputs. Returns (in_maps, plan)."""
    import ml_dtypes

    x = np.ascontiguousarray(np.asarray(x, dtype=np.float32))
    community = np.asarray(community).astype(np.int64)
    n = x.shape[0]
    assert n % M == 0
    nl = n // M

    comm_sh = community.reshape(M, nl)
    perms = np.argsort(comm_sh, axis=1, kind="stable")
    comm_sorted = np.take_along_axis(comm_sh, perms, axis=1)

    # per (core, chunk) node counts
    chunk_ids = comm_sorted >> 7  # // 128
    cnts = np.zeros((M, NC_CHUNKS), dtype=np.int64)
    for m in range(M):
        bc = np.bincount(chunk_ids[m], minlength=NC_CHUNKS)
        cnts[m] = bc[:NC_CHUNKS]
    t_k = np.maximum(1, -(-cnts.max(axis=0) // P))  # ceil, shared by all cores
    t_total = int(t_k.sum())
    chunk_of_tile = np.repeat(np.arange(NC_CHUNKS), t_k)
    tile_off = np.concatenate([[0], np.cumsum(t_k)])  # tile index base per chunk

    # counts -> 1/max(cnt, eps), [p, k] layout (community id = k*128 + p)
    cnt_full = np.bincount(community, minlength=NUM_COMMS).astype(np.float32)
    inv_pad = np.zeros((NC_CHUNKS * P,), np.float32)
    inv_pad[:NUM_COMMS] = 1.0 / np.maximum(cnt_full, np.float32(EPS))
    invc = np.ascontiguousarray(inv_pad.reshape(NC_CHUNKS, P).T)  # [128, 8]

    iota = np.ascontiguousarray(
        np.tile(np.arange(P, dtype=ml_dtypes.bfloat16), (P, 1))
    )  # [128, 128], each row 0..127

    in_maps = []
    origs = []
    for m in range(M):
        x_m = x[m * nl : (m + 1) * nl]
        locid = np.full((t_total * P,), -1.0, dtype=np.float32)
        orig = np.full((t_total * P,), -1, dtype=np.int64)
        start = 0
        for k in range(NC_CHUNKS):
            c = int(cnts[m, k])
            row = int(tile_off[k]) * P
            sel = perms[m, start : start + c]
            orig[row : row + c] = sel
            locid[row : row + c] = comm_sorted[m, start : start + c] - k * P
            start += c
        # partition-major [128, T] index grid; gather x directly into
        # [128, T, F] so the device reads contiguous per-partition lines
        orig_t = np.ascontiguousarray(orig.reshape(t_total, P).T)  # [128, T]
        xs = x_m[np.maximum(orig_t, 0)]  # [128, T, F] fp32
        xs[orig_t < 0] = 0.0
        xs_t = np.ascontiguousarray(
            xs.astype(ml_dtypes.bfloat16).reshape(P, t_total * F)
        )
        locid_t = np.ascontiguousarray(locid.reshape(t_total, P).T)  # [128, T] fp32

        # fp8 one-hot transpose: bt8[c, t*128+p] = 1 iff locid[t*128+p] == c
        bt8 = np.zeros((P, t_total * P), dtype=ml_dtypes.float8_e4m3fn)
        nidx = np.arange(t_total * P)
        valid = orig >= 0
        bt8[locid[valid].astype(np.int64), nidx[valid]] = 1.0

        origs.append(orig_t)
        in_maps.append(
            {
                "xs": xs_t,
                "locid": locid_t,
                "bt8": bt8,
                "iota": iota,
                "invc": invc,
            }
        )

    plan = {
        "nl": nl,
        "t_k": [int(v) for v in t_k],
        "t_total": t_total,
        "chunk_of_tile": [int(v) for v in chunk_of_tile],
        "tile_off": [int(v) for v in tile_off],
        "origs": origs,
    }
    return in_maps, plan


def _build_program(plan, use_collective=True):
    from concourse import bacc, mybir, tile

    t_total = plan["t_total"]
    t_k = plan["t_k"]
    chunk_of_tile = plan["chunk_of_tile"]
    tile_off = plan["tile_off"]
    max_tk = max(t_k)
    HALF = NC_CHUNKS // 2
    h0_end = tile_off[HALF]  # first tile of chunk 4

    dt = mybir.dt
    nc = bacc.Bacc("TRN2", target_bir_lowering=False, debug=False, num_devices=M)

    xs = nc.dram_tensor("xs", [P, t_total * F], dt.bfloat16, kind="ExternalInput")
    locid = nc.dram_tensor("locid", [P, t_total], dt.float32, kind="ExternalInput")
    bt8 = nc.dram_tensor("bt8", [P, t_total * P], dt.float8e4, kind="ExternalInput")
    iota = nc.dram_tensor("iota", [P, P], dt.bfloat16, kind="ExternalInput")
    invc = nc.dram_tensor("invc", [P, NC_CHUNKS], dt.float32, kind="ExternalInput")
    out = nc.dram_tensor("out", [P, t_total * F], dt.bfloat16, kind="ExternalOutput")

    with tile.TileContext(nc) as tc:
        with (
            tc.tile_pool(name="const", bufs=1) as constp,
            tc.tile_pool(name="xsp", bufs=3) as xsp,
            tc.tile_pool(name="bp", bufs=4) as bp,
            tc.tile_pool(name="acc", bufs=1) as accp,
            tc.tile_pool(name="btp", bufs=2) as btp,
            tc.tile_pool(name="dram", bufs=1, space="DRAM") as dramp,
        ):
            iota_t = constp.tile([P, P], dt.bfloat16)
            nc.sync.dma_start(out=iota_t[:], in_=iota.ap())
            locid_t = constp.tile([P, t_total], dt.float32)
            nc.sync.dma_start(out=locid_t[:], in_=locid.ap())
            invc_t = constp.tile([P, NC_CHUNKS], dt.float32)
            nc.sync.dma_start(out=invc_t[:], in_=invc.ap())

            comm_sum = accp.tile([P, NC_CHUNKS * F], dt.float32)
            mean_f32 = accp.tile([P, NC_CHUNKS * F], dt.float32)
            mean_bf = accp.tile([P, NC_CHUNKS * F], dt.bfloat16)
            HF = HALF * F  # columns per AR half
            ar_in0 = dramp.tile([P, HF], dt.float32)
            ar_in1 = dramp.tile([P, HF], dt.float32)
            ar_out0 = dramp.tile([P, HF], dt.float32)
            ar_out1 = dramp.tile([P, HF], dt.float32)
            ar_in = [ar_in0, ar_in1]
            ar_out = [ar_out0, ar_out1]

            def emit_ar_half(h):
                """All-reduce one half of comm_sum, then scale by 1/count.

                Queue placement is load-bearing: AR input dmas ride the
                GpSimd SWDGE queue (idle in phase 1) so neither HWDGE
                ring's FIFO serializes AR1 behind AR0's output dma.
                Half 0's output chain sits on Scalar (fast scale ->
                phase-2 chunk 0 starts right after AR0); half 1's rides
                GpSimd where its latency hides behind phase-2 chunks 0-3.
                """
                lo, hi = h * HF, (h + 1) * HF
                nc.gpsimd.dma_start(out=ar_in[h], in_=comm_sum[:, lo:hi])
                if use_collective:
                    nc.gpsimd.collective_compute(
                        "AllReduce",
                        mybir.AluOpType.add,
                        replica_groups=[list(range(M))],
                        ins=[ar_in[h].opt()],
                        outs=[ar_out[h].opt()],
                    )
                else:
                    nc.gpsimd.dma_start(out=ar_out[h], in_=ar_in[h])
                if h == 0:
                    nc.scalar.dma_start(
                        out=mean_f32[:, lo:hi], in_=ar_out[h]
                    )
                    for k in range(0, HALF):
                        nc.scalar.mul(
                            mean_bf[:, k * F : (k + 1) * F],
                            mean_f32[:, k * F : (k + 1) * F],
                            invc_t[:, k : k + 1],
                        )
                else:
                    nc.gpsimd.dma_start(out=mean_f32[:, lo:hi], in_=ar_out[h])
                    for k in range(HALF, NC_CHUNKS):
                        nc.gpsimd.tensor_scalar(
                            mean_bf[:, k * F : (k + 1) * F],
                            mean_f32[:, k * F : (k + 1) * F],
                            invc_t[:, k : k + 1],
                            None,
                            mybir.AluOpType.mult,
                        )

            # ---- phase 1: streamed one-hot matmul segment sums ----
            with tc.tile_pool(name="ps1", bufs=2, space="PSUM") as ps1p:
                xsb = None
                psum_t = None
                for t in range(t_total):
                    if t % XB == 0:
                        w = min(XB, t_total - t)
                        xsb = xsp.tile([P, XB * F], dt.bfloat16, tag="xsb")
                        nc.sync.dma_start(
                            out=xsb[:, : w * F],
                            in_=xs.ap()[:, t * F : (t + w) * F],
                        )
                    k = chunk_of_tile[t]
                    first = t == tile_off[k]
                    last = t == tile_off[k + 1] - 1
                    if first:
                        psum_t = ps1p.tile([P, F], dt.float32)
                    b_t = bp.tile([P, P], dt.bfloat16, tag="b")
                    nc.vector.tensor_scalar(
                        b_t[:],
                        iota_t[:],
                        locid_t[:, t : t + 1],
                        None,
                        mybir.AluOpType.is_equal,
                    )
                    j = t % XB
                    nc.tensor.matmul(
                        psum_t[:],
                        lhsT=b_t[:],
                        rhs=xsb[:, j * F : (j + 1) * F],
                        start=first,
                        stop=last,
                    )
                    if last:
                        nc.vector.tensor_copy(
                            out=comm_sum[:, k * F : (k + 1) * F], in_=psum_t[:]
                        )
                    if t == h0_end - 1:
                        emit_ar_half(0)
                emit_ar_half(1)

            # ---- phase 2: broadcast means back to (sorted) nodes ----
            # per tile: matmul(lhsT = fp8 one-hot bt, rhs = bf16 mean
            # chunk) -> fp32 PSUM; evacuate in W-tile groups alternating
            # DVE/Scalar; write WG-tile DMAs.
            with (
                tc.tile_pool(name="pso", bufs=4, space="PSUM") as psop,
                tc.tile_pool(name="outp", bufs=3) as outp,
            ):
                gidx = 0
                for k in range(NC_CHUNKS):
                    k0, k1 = tile_off[k], tile_off[k + 1]
                    bt_sb = btp.tile([P, max_tk * P], dt.float8e4, tag="btc")
                    nc.sync.dma_start(
                        out=bt_sb[:, : (k1 - k0) * P],
                        in_=bt8.ap()[:, k0 * P : k1 * P],
                    )
                    outsb = None
                    wg0 = k0
                    for g0 in range(k0, k1, W):
                        g1 = min(g0 + W, k1)
                        w = g1 - g0
                        if g0 == wg0:
                            outsb = outp.tile([P, WG * F], dt.bfloat16,
                                              tag="outsb")
                        pso_t = psop.tile([P, W * F], dt.float32, tag="pso")
                        for j in range(w):
                            t = g0 + j
                            nc.tensor.matmul(
                                pso_t[:, j * F : (j + 1) * F],
                                lhsT=bt_sb[:, (t - k0) * P : (t - k0 + 1) * P],
                                rhs=mean_bf[:, k * F : (k + 1) * F],
                                start=True,
                                stop=True,
                            )
                        c0 = (g0 - wg0) * F
                        if gidx % 2 == 0:
                            nc.vector.tensor_copy(
                                out=outsb[:, c0 : c0 + w * F],
                                in_=pso_t[:, : w * F],
                            )
                        else:
                            nc.scalar.copy(
                                out=outsb[:, c0 : c0 + w * F],
                                in_=pso_t[:, : w * F],
                            )
                        gidx += 1
                        if g1 - wg0 >= WG or g1 == k1:
                            nc.sync.dma_start(
                                out=out.ap()[:, wg0 * F : g1 * F],
                                in_=outsb[:, : (g1 - wg0) * F],
                            )
                            wg0 = g1

    nc.compile()
    return nc


def kernel(x, community):
    global LAST_RESULTS
    from concourse.bass_utils import run_bass_kernel_spmd

    in_maps, plan = _host_prep(x, community)
    nc = _build_program(plan)
    res = run_bass_kernel_spmd(nc, in_maps, core_ids=list(range(M)))
    LAST_RESULTS = res
    nl = plan["nl"]
    t_total = plan["t_total"]
    outs = []
    for m in range(M):
        out_t = np.asarray(res.results[m]["out"]).reshape(P, t_total, F)
        orig_t = plan["origs"][m]  # [128, T]
        valid = orig_t >= 0
        out_m = np.empty((nl, F), dtype=np.float32)
        out_m[orig_t[valid]] = out_t[valid].astype(np.float32)
        outs.append(out_m)
    return np.concatenate(outs, axis=0)
